# revision 53
# baseline (speedup 1.0000x reference)
"""ANI-style AEV computer (radial + angular) on 8 Trainium2 NeuronCores.

Strategy
--------
Data-parallel over molecules (32/core), with host-side *indexing only*
(neighborlists / triple lists / one-hot bin matrices); every floating-point
operation of the AEV math runs on-device.

Angular part: the host enumerates surviving triples (center i, neighbors
j<k within Rca) into a flat per-core list, sorted by (molecule-slot,
center-half, center, species-pair-bin).  The device computes, per
128-triple chunk column: geometry (vectors, d^2, dot via grouped X-reduce,
j/k batched) -> 1/d and d via ACT ln/exp -> cos/sin of the angle (sin via
ln/exp of 1-cos^2) -> cutoff poly -> f2 = exp(-eta/4 (dsum-2shf)^2) ->
q8 via the angle-sum identity (4 shifts + mirrored 4) -> f1 = q^zeta via
exp(zeta*ln q) -> G = w*f2 (x) f1 [bf16, 32 features], binned into
(center, species-pair) segments by PE matmuls against fp8 one-hot
matrices (PSUM-accumulated across a segment-group's chunks; 5 psum-group
blocks of descending size trail the DVE chain through PE -> ACT copy ->
DMA out).

Radial part: a within-Rcr pair list chunked the same way (groups =
molecule slots, chunk-aligned so every group's chunks are private);
segment = center*4 + species(j), 96 segments padded to a 128-wide fp8
one-hot so LDWEIGHTS uses the fast quadrant path.  Pair matmuls land in
two alternating psum banks (even/odd slots).  The radial shift expansion
rides the ACT slack between the angular activations.

Performance notes: one ACT table set (natural_log_exp) is used - cutoff
cosines are a degree-3 Chebyshev polynomial in u^2 (error ~1e-5), both
cutoffs share one merged Horner chain; outputs are staged/DMAed as bf16
and upcast on host; the GEO DMA streams in two halves ahead of the big
one-hots so the DVE geometry starts as early as possible.
"""

import os
import sys

import numpy as np

for _p in ("/opt/trn_rl_repo", "/root/.axon_site/_ro/trn_rl_repo"):
    if os.path.isdir(_p) and _p not in sys.path:
        sys.path.insert(0, _p)

import concourse.bass as bass
import concourse.mybir as mybir
from concourse import bacc, tile
from concourse.bass_utils import run_bass_kernel_spmd

import ml_dtypes

AF = mybir.ActivationFunctionType
ALU = mybir.AluOpType
AX = mybir.AxisListType
dt = mybir.dt
AP = bass.AP

FP8 = ml_dtypes.float8_e4m3

# ---- hyperparameters (match reference) ----
NCORES = 8
M, A = 256, 24
MLOC = M // NCORES          # 32 molecules per core
RCR, RCA = 5.2, 3.5
ETA_R, ETA_A, ZETA = 16.0, 8.0, 32.0
SHF_R = np.linspace(0.9, 5.2, 17)[:-1].astype(np.float64)   # 16
SHF_A = np.linspace(0.9, 3.5, 5)[:-1].astype(np.float64)    # 4
SHF_Z = (np.arange(8) + 0.5) * np.pi / 8.0                   # 8
NPAIR, RSUB, ASUB = 10, 16, 32
NSEG = 120                  # segments per psum group = 12 centers x 10 bins
GSEG = 128                  # one-hot width (8 pad cols)
NG = 2 * MLOC               # 64 groups/core (2 per molecule slot)
PGRP = 16                   # psum groups packed per PSUM bank tile
RGRP = MLOC // 4            # 8 radial groups of 4 molecules (96 = 4*24 rows)
# device z order: front shifts 0..3, then back shifts (pi - z) = ref 7,6,5,4
ZPERM = np.array([0, 1, 2, 3, 7, 6, 5, 4])

_TRIU = np.zeros((4, 4), np.int64)
_s1, _s2 = np.triu_indices(4)
_TRIU[_s1, _s2] = np.arange(len(_s1))
_TRIU[_s2, _s1] = _TRIU[_s1, _s2]

# ---- degree-3 (in v=u^2) Chebyshev fit of cos(pi*u/2) on u in [0,1]
# (max err ~1.2e-5 -> ~5e-5 on features; negligible vs the bf16 outputs) ----
def _cos_poly():
    v = np.linspace(0.0, 1.0, 4001)
    tgt = np.cos(0.5 * np.pi * np.sqrt(v))
    from numpy.polynomial import chebyshev as C
    ch = C.Chebyshev.fit(v, tgt, 3, domain=[0, 1])
    pw = ch.convert(kind=np.polynomial.Polynomial)
    c = pw.coef  # c0..c3 in v
    K = c[3]
    a = c[:3] / K  # monic residual coeffs a0..a2
    return K, a


_POLY_K, _POLY_A = _cos_poly()

# const tile column map ([128, _C_W] fp32)
_C_SHF2A = 0     # 4  : 2*shf_a
_C_SHFR = 4      # 16 : shf_r
_C_CZH = 20      # 4  : 0.5*cos(shf_z[0:4])
_C_SZH = 24      # 4  : 0.5*sin(shf_z[0:4])
_C_MASK = 28     # 24 : radial i==j mask*100 (valid on partitions 0..95)
_C_ONE = 52      # 1  : 1.0 (ln bias for sin)
_C_F2B = 53      # 1  : angular exp bias ln(2*K^4)
_C_RADB = 54     # 1  : radial exp bias ln(0.25*K^2)
_C_W = 55


def _build_consts():
    ct = np.zeros((128, _C_W), np.float32)
    ct[:, _C_SHF2A:_C_SHF2A + 4] = 2.0 * SHF_A
    ct[:, _C_SHFR:_C_SHFR + 16] = SHF_R
    ct[:, _C_CZH:_C_CZH + 4] = 0.5 * np.cos(SHF_Z[:4])
    ct[:, _C_SZH:_C_SZH + 4] = 0.5 * np.sin(SHF_Z[:4])
    mask = np.zeros((128, 24), np.float32)
    for mb in range(4):
        for j in range(24):
            mask[mb * 24 + j, j] = 100.0
    ct[:, _C_MASK:_C_MASK + 24] = mask
    K = _POLY_K
    ct[:, _C_ONE] = 1.0
    ct[:, _C_F2B] = np.log(2.0) + 4.0 * np.log(abs(K))
    ct[:, _C_RADB] = np.log(0.25) + 2.0 * np.log(abs(K))
    return ct


# ============================================================
# host-side indexing prep (no float math enters the output path)
# ============================================================

def _prep(species, coordinates):
    sp = np.asarray(species)
    co = np.asarray(coordinates, np.float32)
    cod = co.astype(np.float64)
    vec = cod[:, None, :, :] - cod[:, :, None, :]       # [m, i, j, 3] = r_j - r_i
    dmat = np.sqrt(np.maximum((vec ** 2).sum(-1), 0.0))
    adj = (dmat <= RCA) & ~np.eye(A, dtype=bool)[None]

    # per-(m, i) neighbor lists and per-half triple counts
    nbrs = [[np.where(adj[m, i])[0] for i in range(A)] for m in range(M)]
    tri_mi = np.array([[len(nbrs[m][i]) * (len(nbrs[m][i]) - 1) // 2
                        for i in range(A)] for m in range(M)], np.int64)
    Th = np.stack([tri_mi[:, :12].sum(1), tri_mi[:, 12:].sum(1)], 1)  # [M, 2]

    # molecule -> (core, slot): sort by total triples, deal rank-groups of 8
    order = np.argsort(-(Th.sum(1)), kind="stable")
    slot2mol = np.empty((NCORES, MLOC), np.int64)
    for s in range(MLOC):
        for c in range(NCORES):
            slot2mol[c, s] = order[s * NCORES + c]

    # flat per-core triple list (sorted by slot/half/unit/p); no per-group
    # padding -- groups map to chunk RANGES (union over cores), and boundary
    # chunks matmul into more than one psum group.
    SYNCW = 8   # re-align cores to a chunk boundary every SYNCW groups
    tlo = np.zeros((NCORES, NG), np.int64)   # triple range per group
    thi = np.zeros((NCORES, NG), np.int64)
    posv = np.zeros(NCORES, np.int64)
    for g in range(NG):
        s, h = g // 2, g % 2
        if g % SYNCW == 0:
            posv[:] = int(np.ceil(posv.max() / 128.0)) * 128
        tlo[:, g] = posv
        posv += Th[slot2mol[:, s], h]
        thi[:, g] = posv
    nch = int(np.ceil(posv.max() / 128.0))
    # chunk span per group (uniform): union over cores
    clo = np.empty(NG, np.int64)
    chi = np.empty(NG, np.int64)
    for g in range(NG):
        clo[g] = (tlo[:, g] // 128).min()
        hi = np.maximum(thi[:, g] - 1, tlo[:, g]) // 128
        chi[g] = max(hi.max(), clo[g])
    span = (chi - clo + 1).astype(np.int64)
    mm_base = np.concatenate([[0], np.cumsum(span)])
    n_mm = int(mm_base[-1])

    # packed geometry: [:, :, 0:3]=r_j, [:, :, 3:6]=r_k, [:, :, 6:9]=r_i
    geo = np.zeros((NCORES, 128, nch, 9), np.float32)
    oh = np.zeros((NCORES, 128, n_mm, GSEG), FP8)

    for c in range(NCORES):
        def put_pad(a, b, mref):
            if a >= b:
                return
            t_idx = np.arange(a, b)
            chs, ts = t_idx // 128, t_idx % 128
            geo[c, ts, chs, 0:3] = mref + np.array([50, 0, 0], np.float32)
            geo[c, ts, chs, 3:6] = mref + np.array([0, 50, 0], np.float32)
            geo[c, ts, chs, 6:9] = mref
        prev_end = 0
        for s in range(MLOC):
            m = slot2mol[c, s]
            for h in range(2):
                g = 2 * s + h
                put_pad(prev_end, tlo[c, g], co[m, 0])  # sync-pad gap
                pos = tlo[c, g]
                for u in range(12):
                    i = h * 12 + u
                    nb = nbrs[m][i]
                    if len(nb) < 2:
                        continue
                    jj, kk = np.triu_indices(len(nb), 1)
                    j, k = nb[jj], nb[kk]
                    p = _TRIU[sp[m, j], sp[m, k]]
                    o = np.argsort(p, kind="stable")
                    j, k, p = j[o], k[o], p[o]
                    n = len(j)
                    t_idx = np.arange(pos, pos + n)
                    chs, ts = t_idx // 128, t_idx % 128
                    geo[c, ts, chs, 0:3] = co[m, j]
                    geo[c, ts, chs, 3:6] = co[m, k]
                    geo[c, ts, chs, 6:9] = np.broadcast_to(co[m, i], (n, 3))
                    oh[c, ts, mm_base[g] + chs - clo[g], p * 12 + u] = 1
                    pos += n
                prev_end = pos
        put_pad(prev_end, nch * 128, co[slot2mol[c, 0], 0])

    # ---- radial inputs: pair-list (only pairs within Rcr), chunked like
    # the angular triples.  Groups = molecule slots (32); segment within a
    # group = i*4 + species(j) (96 of them); boundary chunks matmul into
    # two psum column ranges.
    adjr = (dmat <= RCR) & ~np.eye(A, dtype=bool)[None]
    prc = adjr.sum((1, 2))                     # pairs per molecule
    tlo_r = np.zeros((NCORES, MLOC), np.int64)
    thi_r = np.zeros((NCORES, MLOC), np.int64)
    posv = np.zeros(NCORES, np.int64)
    SYNCR = 1
    for g in range(MLOC):
        if g % SYNCR == 0:
            posv[:] = int(np.ceil(posv.max() / 128.0)) * 128
        tlo_r[:, g] = posv
        posv += prc[slot2mol[:, g]]
        thi_r[:, g] = posv
    nchr = int(np.ceil(posv.max() / 128.0))
    clo_r = np.empty(MLOC, np.int64)
    chi_r = np.empty(MLOC, np.int64)
    for g in range(MLOC):
        clo_r[g] = (tlo_r[:, g] // 128).min()
        hi = np.maximum(thi_r[:, g] - 1, tlo_r[:, g]) // 128
        chi_r[g] = max(hi.max(), clo_r[g])
    span_r = (chi_r - clo_r + 1).astype(np.int64)
    mmb_r = np.concatenate([[0], np.cumsum(span_r)])
    n_mmr = int(mmb_r[-1])

    geor = np.zeros((NCORES, 128, nchr, 6), np.float32)
    ohr = np.zeros((NCORES, 128, n_mmr, 128), FP8)
    for c in range(NCORES):
        def put_pad_r(a, b, mref):
            if a >= b:
                return
            t_idx = np.arange(a, b)
            chs, ts = t_idx // 128, t_idx % 128
            geor[c, ts, chs, 0:3] = mref + np.array([50, 0, 0], np.float32)
            geor[c, ts, chs, 3:6] = mref
        prev_end = 0
        for g in range(MLOC):
            m = slot2mol[c, g]
            put_pad_r(prev_end, tlo_r[c, g], co[m, 0])
            pos = tlo_r[c, g]
            ii, jj = np.nonzero(adjr[m])
            n = len(ii)
            t_idx = np.arange(pos, pos + n)
            chs, ts = t_idx // 128, t_idx % 128
            geor[c, ts, chs, 0:3] = co[m, jj]
            geor[c, ts, chs, 3:6] = co[m, ii]
            ohr[c, ts, mmb_r[g] + chs - clo_r[g], ii * 4 + sp[m, jj]] = 1
            pos += n
            prev_end = pos
        put_pad_r(prev_end, nchr * 128, co[slot2mol[c, 0], 0])

    meta = dict(nch=nch, n_mm=n_mm, clo=tuple(int(x) for x in clo),
                chi=tuple(int(x) for x in chi), slot2mol=slot2mol,
                nchr=nchr, n_mmr=n_mmr,
                clo_r=tuple(int(x) for x in clo_r),
                chi_r=tuple(int(x) for x in chi_r))
    arrays = dict(geo=geo, oh=oh, geor=geor, ohr=ohr)
    return meta, arrays


# ============================================================
# device program
# ============================================================

def _bb(ap, dims, off=0):
    """Build a broadcast/strided view: keep ap's partition dim, replace free
    dims with explicit [step, count] pairs (element units)."""
    return AP(ap.tensor, ap.offset + off,
              [list(ap.ap[0])] + [list(d) for d in dims])


def _build(nch, clo, chi, nchr, clo_r, chi_r):
    span = [chi[g] - clo[g] + 1 for g in range(NG)]
    mm_base = [0]
    for g in range(NG):
        mm_base.append(mm_base[-1] + span[g])
    n_mm = mm_base[-1]
    nb = nch
    span_r = [chi_r[g] - clo_r[g] + 1 for g in range(MLOC)]
    mmb_r = [0]
    for g in range(MLOC):
        mmb_r.append(mmb_r[-1] + span_r[g])
    n_mmr = mmb_r[-1]
    nbr = nchr

    nc = bacc.Bacc(None, target_bir_lowering=False)
    geo_d = nc.declare_dram_parameter("geo", [128, nch, 9], dt.float32, False)
    oh_d = nc.declare_dram_parameter("oh", [128, n_mm, GSEG], dt.float8e4,
                                     False)
    geor_d = nc.declare_dram_parameter("geor", [128, nchr, 6], dt.float32,
                                       False)
    ohr_d = nc.declare_dram_parameter("ohr", [128, n_mmr, 128], dt.float8e4,
                                      False)
    ct_d = nc.declare_dram_parameter("consts", [128, _C_W], dt.float32, False)
    outa_d = nc.declare_dram_parameter("outa", [GSEG, NG * 32], dt.bfloat16,
                                       True)
    outr_d = nc.declare_dram_parameter("outr", [128, MLOC * 16], dt.bfloat16,
                                       True)

    K, a = _POLY_K, _POLY_A
    # fold 2*K^4 (w = 2*fc_j*fc_k = 2*(K^2 s4j^2)(K^2 s4k^2)) into f2's exp bias
    F2BIAS = float(np.log(2.0) + 4.0 * np.log(abs(K)))
    # radial: rad = 0.25 * fc * exp(...) ; fc = (K*s4)^2
    RADBIAS = float(np.log(0.25) + 2.0 * np.log(abs(K)))
    f32, bf16 = dt.float32, dt.bfloat16

    with tile.TileContext(nc) as tc:
        with (
            tc.tile_pool(name="const", bufs=1) as cpool,
            tc.tile_pool(name="io", bufs=1) as io,
            tc.tile_pool(name="geo", bufs=1) as gp_,
            tc.tile_pool(name="feat", bufs=1) as feat,
            tc.tile_pool(name="stg", bufs=1) as stg,
            tc.tile_pool(name="gt", bufs=5) as gtp,
            tc.tile_pool(name="ps", bufs=5, space="PSUM") as ps,
            tc.tile_pool(name="psr", bufs=1, space="PSUM") as psr,
        ):
            CT = cpool.tile([128, _C_W], dt.float32)

            AZSTG = stg.tile([GSEG, NG * 32], bf16)   # angular staging
            RDSTG = stg.tile([128, MLOC * 16], bf16)  # radial staging

            V = nc.vector
            S = nc.scalar

            # DMA order: angular geometry first (gates the long chain), the
            # radial pair list second, then the big one-hots (matmul phase).
            GEO = io.tile([128, 9 * nch], f32, tag="geo")
            nh0 = nch // 2
            nc.sync.dma_start(
                GEO[:, :9 * nh0].rearrange("p (n c) -> p n c", c=9),
                geo_d[:, :nh0])
            nc.sync.dma_start(
                GEO[:, 9 * nh0:].rearrange("p (n c) -> p n c", c=9),
                geo_d[:, nh0:])
            GEOR = io.tile([128, 6 * nchr], f32, tag="geor")
            nc.scalar.dma_start(
                GEOR[:].rearrange("p (n c) -> p n c", c=6), geor_d[:])
            nc.scalar.dma_start(CT[:], ct_d[:])
            OHR = io.tile([128, 128 * n_mmr], dt.float8e4, tag="ohr")
            nc.gpsimd.dma_start(
                OHR[:].rearrange("p (n s) -> p n s", s=128), ohr_d[:])
            OHL = io.tile([128, GSEG * n_mm], dt.float8e4, tag="oh")
            nc.gpsimd.dma_start(
                OHL[:].rearrange("p (n s) -> p n s", s=GSEG), oh_d[:])

            def poly_fc(E, dist, n, tag, rc, npart=128, stt=True):
                """fc up to factor K^2: returns s4 with fc = (K*s4)^2."""
                u = gp_.tile([npart, n], f32, tag=tag + "_u")
                E.tensor_scalar(u[:], dist[:], rc, 1.0 / rc, ALU.min,
                                ALU.mult)
                v = gp_.tile([npart, n], f32, tag=tag + "_v")
                E.tensor_tensor(v[:], u[:], u[:], ALU.mult)
                acc = gp_.tile([npart, n], f32, tag=tag + "_acc")
                if stt:
                    E.scalar_tensor_tensor(acc[:], v[:], float(a[3]), v[:],
                                           ALU.add, ALU.mult)
                    E.scalar_tensor_tensor(acc[:], acc[:], float(a[2]), v[:],
                                           ALU.add, ALU.mult)
                    E.scalar_tensor_tensor(acc[:], acc[:], float(a[1]), v[:],
                                           ALU.add, ALU.mult)
                else:
                    # gpsimd lacks scalar_tensor_tensor: Horner via TS+TT
                    E.tensor_scalar(acc[:], v[:], float(a[3]), None, ALU.add)
                    E.tensor_tensor(acc[:], acc[:], v[:], ALU.mult)
                    E.tensor_scalar(acc[:], acc[:], float(a[2]), None,
                                    ALU.add)
                    E.tensor_tensor(acc[:], acc[:], v[:], ALU.mult)
                    E.tensor_scalar(acc[:], acc[:], float(a[1]), None,
                                    ALU.add)
                    E.tensor_tensor(acc[:], acc[:], v[:], ALU.mult)
                E.tensor_scalar(acc[:], acc[:], float(a[0]), None, ALU.add)
                return acc

            # ---------------- angular geometry (j/k batched) ---------------
            VJK = gp_.tile([128, 6 * nb], f32, tag="vjk")
            SQ = gp_.tile([128, 6 * nb], f32, tag="sq")
            D2JK = gp_.tile([128, 2 * nb], f32, tag="d2jk")
            DOTP = gp_.tile([128, 3 * nb], f32, tag="dotp")
            DOT = gp_.tile([128, nb], f32, tag="dot")
            V.tensor_tensor(
                _bb(VJK[:], [[6, nb], [3, 2], [1, 3]]),
                _bb(GEO[:], [[9, nb], [3, 2], [1, 3]]),
                _bb(GEO[:], [[9, nb], [0, 2], [1, 3]], off=6),
                ALU.subtract)
            V.tensor_tensor(SQ[:], VJK[:], VJK[:], ALU.mult)
            V.tensor_reduce(D2JK[:], _bb(SQ[:], [[3, 2 * nb], [1, 3]]),
                            AX.X, ALU.add)
            V.tensor_tensor(
                _bb(DOTP[:], [[3, nb], [1, 3]]),
                _bb(VJK[:], [[6, nb], [1, 3]], off=0),
                _bb(VJK[:], [[6, nb], [1, 3]], off=3),
                ALU.mult)
            V.tensor_reduce(DOT[:], _bb(DOTP[:], [[3, nb], [1, 3]]),
                            AX.X, ALU.add)

            # ---- radial pair geometry fills DVE while ACT runs ln/exp ----
            VR = gp_.tile([128, 3 * nbr], f32, tag="vr")
            V.tensor_tensor(
                _bb(VR[:], [[3, nbr], [1, 3]]),
                _bb(GEOR[:], [[6, nbr], [1, 3]], off=0),
                _bb(GEOR[:], [[6, nbr], [1, 3]], off=3),
                ALU.subtract)
            S.activation(VR[:], VR[:], AF.Square)
            RD2 = gp_.tile([128, nbr], f32, tag="rd2")
            V.tensor_reduce(RD2[:], _bb(VR[:], [[3, nbr], [1, 3]]),
                            AX.X, ALU.add)

            # d, 1/d via single ln + two exps (one ACT table set total)
            LNJK = gp_.tile([128, 2 * nb], f32, tag="lnjk")
            DJK = gp_.tile([128, 2 * nb], f32, tag="djk")
            RJK = gp_.tile([128, 2 * nb], f32, tag="rjk")
            S.activation(LNJK[:], D2JK[:], AF.Ln)
            S.activation(DJK[:], LNJK[:], AF.Exp, scale=0.5)
            S.activation(RJK[:], LNJK[:], AF.Exp, scale=-0.5)
            RD = gp_.tile([128, nbr], f32, tag="rdist")
            S.activation(RD[:], RD2[:], AF.Ln)
            S.activation(RD[:], RD[:], AF.Exp, scale=0.5)

            COS = gp_.tile([128, nb], f32, tag="cos")
            V.scalar_tensor_tensor(COS[:], DOT[:], 0.95,
                                   _bb(RJK[:], [[2, nb]], off=0),
                                   ALU.mult, ALU.mult)
            V.tensor_tensor(COS[:], COS[:],
                            _bb(RJK[:], [[2, nb]], off=1), ALU.mult)
            CS2 = gp_.tile([128, nb], f32, tag="cs2")
            S.activation(CS2[:], COS[:], AF.Square)
            SIN = gp_.tile([128, nb], f32, tag="sin")
            # sin = exp(0.5*ln(1 - cos^2))
            S.activation(SIN[:], CS2[:], AF.Ln, scale=-1.0,
                         bias=CT[:, _C_ONE:_C_ONE + 1])
            S.activation(SIN[:], SIN[:], AF.Exp, scale=0.5)

            # radial shift expansion in two chunk-halves (split at the
            # group-16 chunk boundary) so DVE/ACT/PE pipeline
            GH = MLOC // 2
            hc0 = chi_r[GH - 1] + 1
            RT = feat.tile([128, 16 * nbr], f32, tag="rt")

            def rad_rt(h):
                lo = 0 if h == 0 else hc0
                hi = hc0 if h == 0 else nbr
                V.tensor_tensor(
                    _bb(RT[:], [[16, hi - lo], [1, 16]], off=16 * lo),
                    _bb(RD[:], [[1, hi - lo], [0, 16]], off=lo),
                    _bb(CT[:, _C_SHFR:], [[0, hi - lo], [1, 16]]),
                    ALU.subtract)
                hs = slice(16 * lo, 16 * hi)
                S.activation(RT[:, hs], RT[:, hs], AF.Square)
                S.activation(RT[:, hs], RT[:, hs], AF.Exp, scale=-ETA_R,
                             bias=CT[:, _C_RADB:_C_RADB + 1])

            # radial square/exp fills the ACT slack before the angular
            # feature activations
            rad_rt(0)
            rad_rt(1)

            # merged cutoff poly: u for both cutoffs side by side, then one
            # Horner chain over [2nb + nbr] (5 big ops instead of 10)
            NPOLY = 2 * nb + nbr
            PU = gp_.tile([128, NPOLY], f32, tag="pu")
            V.tensor_scalar(PU[:, :2 * nb], DJK[:], RCA, 1.0 / RCA, ALU.min,
                            ALU.mult)
            V.tensor_scalar(PU[:, 2 * nb:], RD[:], RCR, 1.0 / RCR, ALU.min,
                            ALU.mult)
            PV = gp_.tile([128, NPOLY], f32, tag="pv")
            V.tensor_tensor(PV[:], PU[:], PU[:], ALU.mult)
            PA = gp_.tile([128, NPOLY], f32, tag="pa")
            V.scalar_tensor_tensor(PA[:], PV[:], float(a[2]), PV[:],
                                   ALU.add, ALU.mult)
            V.scalar_tensor_tensor(PA[:], PA[:], float(a[1]), PV[:],
                                   ALU.add, ALU.mult)
            V.tensor_scalar(PA[:], PA[:], float(a[0]), None, ALU.add)
            S4 = PA[:, :2 * nb]
            RFC = gp_.tile([128, nbr], f32, tag="rfc")
            V.tensor_tensor(RFC[:], PA[:, 2 * nb:], PA[:, 2 * nb:],
                            ALU.mult)
            RAD = feat.tile([128, 16 * nbr], bf16, tag="rad")

            def rad_fill(h):
                lo = 0 if h == 0 else hc0
                hi = hc0 if h == 0 else nbr
                V.tensor_tensor(
                    _bb(RAD[:], [[16, hi - lo], [1, 16]], off=16 * lo),
                    _bb(RFC[:], [[1, hi - lo], [0, 16]], off=lo),
                    _bb(RT[:], [[16, hi - lo], [1, 16]], off=16 * lo),
                    ALU.mult)

            rad_fill(0)
            rad_fill(1)
            W2 = gp_.tile([128, nb], f32, tag="w2")
            V.tensor_tensor(W2[:], _bb(PA[:], [[2, nb]], off=0),
                            _bb(PA[:], [[2, nb]], off=1), ALU.mult)
            V.tensor_tensor(W2[:], W2[:], W2[:], ALU.mult)

            USUM = gp_.tile([128, nb], f32, tag="usum")
            V.tensor_tensor(USUM[:], _bb(DJK[:], [[2, nb]], off=0),
                            _bb(DJK[:], [[2, nb]], off=1), ALU.add)

            # f2[a] = exp(-eta/4*(u - 2shf_a)^2 + F2BIAS), layout (n, a)
            T4 = feat.tile([128, 4 * nb], f32, tag="t4")
            V.tensor_tensor(
                _bb(T4[:], [[4, nb], [1, 4]]),
                _bb(USUM[:], [[1, nb], [0, 4]]),
                _bb(CT[:, _C_SHF2A:], [[0, nb], [1, 4]]),
                ALU.subtract)
            S.activation(T4[:], T4[:], AF.Square)
            S.activation(T4[:], T4[:], AF.Exp, scale=-ETA_A / 4.0,
                         bias=CT[:, _C_F2B:_C_F2B + 1])
            WF2 = feat.tile([128, 4 * nb], bf16, tag="wf2")
            V.tensor_tensor(
                _bb(WF2[:], [[4, nb], [1, 4]]),
                _bb(W2[:], [[1, nb], [0, 4]]),
                _bb(T4[:], [[4, nb], [1, 4]]),
                ALU.mult)

            # q8: q_z = 0.5 + cos*czh_z + sin*szh_z for z=0..3 and the
            # mirrored shifts (pi - z): 0.5 - cos*czh_z + sin*szh_z
            A4 = feat.tile([128, 4 * nb], f32, tag="a4")
            B4 = feat.tile([128, 4 * nb], f32, tag="b4")
            V.tensor_tensor(
                _bb(A4[:], [[4, nb], [1, 4]]),
                _bb(COS[:], [[1, nb], [0, 4]]),
                _bb(CT[:, _C_CZH:], [[0, nb], [1, 4]]),
                ALU.mult)
            V.tensor_tensor(
                _bb(B4[:], [[4, nb], [1, 4]]),
                _bb(SIN[:], [[1, nb], [0, 4]]),
                _bb(CT[:, _C_SZH:], [[0, nb], [1, 4]]),
                ALU.mult)
            Q8 = feat.tile([128, 8 * nb], f32, tag="q8")
            V.scalar_tensor_tensor(
                _bb(Q8[:], [[8, nb], [1, 4]], off=0),
                _bb(A4[:], [[4, nb], [1, 4]]), 0.5,
                _bb(B4[:], [[4, nb], [1, 4]]),
                ALU.add, ALU.add)
            V.scalar_tensor_tensor(
                _bb(Q8[:], [[8, nb], [1, 4]], off=4),
                _bb(B4[:], [[4, nb], [1, 4]]), 0.5,
                _bb(A4[:], [[4, nb], [1, 4]]),
                ALU.add, ALU.subtract)
            # split ln/exp at the first psum-block chunk boundary so the
            # first Gt build can start before the full-width exp finishes
            hcq = 8 * (chi[PGRP - 1] + 1 - clo[0])
            F1 = feat.tile([128, 8 * nb], bf16, tag="f1")
            S.activation(Q8[:, :hcq], Q8[:, :hcq], AF.Ln)
            S.activation(F1[:, :hcq], Q8[:, :hcq], AF.Exp, scale=float(ZETA))
            S.activation(Q8[:, hcq:], Q8[:, hcq:], AF.Ln)
            S.activation(F1[:, hcq:], Q8[:, hcq:], AF.Exp, scale=float(ZETA))

            # ---- angular binning + radial expansion, interleaved so the
            # radial DVE/ACT work pipelines with Gt builds / PE / copies ----
            c0 = clo[0]
            gwmax = max(chi[min(gt + PGRP, NG) - 1] - clo[gt] + 1
                        for gt in range(0, NG, PGRP))
            # two psum banks (even/odd groups) + round-robin chunk order so
            # consecutive matmuls never hit the same accumulation region
            rptA = psr.tile([128, MLOC * 8], dt.float32, tag="rpsA")
            rptB = psr.tile([128, MLOC * 8], dt.float32, tag="rpsB")

            def rad_mm(h):
                for g in range(h * GH, (h + 1) * GH):
                    rp = rptA if g % 2 == 0 else rptB
                    q = 16 * (g // 2)
                    for k in range(span_r[g]):
                        cc = clo_r[g] + k
                        nc.tensor.matmul(
                            rp[:, q:q + 16],
                            OHR[:, 128 * (mmb_r[g] + k):
                                 128 * (mmb_r[g] + k + 1)],
                            RAD[:, 16 * cc:16 * (cc + 1)],
                            start=(k == 0), stop=(k == span_r[g] - 1))
                if h == 1:
                    S.activation(RDSTG[:, :MLOC * 8], rptA[:], AF.Copy)
                    S.activation(RDSTG[:, MLOC * 8:], rptB[:], AF.Copy)
                    nc.sync.dma_start(outr_d[:], RDSTG[:])

            def gt_block(gt, glen=PGRP):
                gl = min(gt + glen, NG)
                ca, cb = clo[gt] - c0, chi[gl - 1] + 1 - c0
                Gt = gtp.tile([128, 32 * gwmax], bf16, tag="G")
                V.tensor_tensor(
                    _bb(Gt[:], [[32, cb - ca], [8, 4], [1, 8]]),
                    _bb(WF2[:, 4 * ca:], [[4, cb - ca], [1, 4], [0, 8]]),
                    _bb(F1[:, 8 * ca:], [[8, cb - ca], [0, 4], [1, 8]]),
                    ALU.mult)
                pt = ps.tile([GSEG, 32 * PGRP], dt.float32, tag="ps")
                pt = pt[:, :32 * glen]
                for g in range(gt, gl):
                    gi = g - gt
                    for k in range(span[g]):
                        cc = clo[g] + k           # absolute chunk
                        nc.tensor.matmul(
                            pt[:, 32 * gi:32 * (gi + 1)],
                            OHL[:, GSEG * (mm_base[g] + k):
                                  GSEG * (mm_base[g] + k + 1)],
                            Gt[:, 32 * (cc - c0 - ca):
                                  32 * (cc - c0 - ca + 1)],
                            start=(k == 0), stop=(k == span[g] - 1))
                sl = slice(32 * gt, 32 * gl)
                S.activation(AZSTG[:, sl], pt[:], AF.Copy)
                nc.sync.dma_start(outa_d[:, sl], AZSTG[:, sl])

            rad_mm(0)
            rad_mm(1)
            gt_block(0)
            gt_block(16)
            gt_block(32, 16)
            gt_block(48, 10)
            gt_block(58, 6)

    _patch_act_tables()
    nc.compile()
    return nc


_ACT_PATCHED = False


def _patch_act_tables():
    """Make Ln/Exp resolve only to the combined natural_log_exp set, so the
    table-load pass emits ONE load instead of thrashing between the ln-only
    and exp-only sets (1.28us per reload)."""
    global _ACT_PATCHED
    if _ACT_PATCHED:
        return
    orig = bacc.get_activation_tables

    def patched(arch):
        t = dict(orig(arch))
        out = {}
        for name, fns in t.items():
            if name != "natural_log_exp_and_others":
                fns = {f for f in fns if f not in (AF.Ln, AF.Exp)}
            out[name] = fns
        return out

    bacc.get_activation_tables = patched
    _ACT_PATCHED = True


_CACHE = {}


def _unpack(res, meta):
    out = np.empty((M, A, 384), np.float32)
    for c in range(NCORES):
        outa = np.asarray(res[c]["outa"]).astype(np.float32)  # [128, NG*32]
        outr = np.asarray(res[c]["outr"]).astype(np.float32)[:96]
        ang = outa.reshape(GSEG, NG, 32)[:120]
        ang = ang.reshape(10, 12, MLOC, 2, 4, 8)        # [p, u, s, h, a, zd]
        ang = ang[..., ZPERM]                           # device z -> ref z
        ang = ang.transpose(2, 3, 1, 0, 4, 5).reshape(MLOC, A, 320)
        # outr rows: seg = i*4 + species(j); cols: [even-slot bank | odd-slot
        # bank], each bank col = (slot//2)*16 + shf
        rad = outr.reshape(A, 4, 2, MLOC // 2, 16)      # [i, sp, ab, gi, r]
        rad = rad.transpose(3, 2, 0, 1, 4).reshape(MLOC, A, 64)
        out[meta["slot2mol"][c], :, :64] = rad
        out[meta["slot2mol"][c], :, 64:] = ang
    return out


def kernel(species, coordinates, coefficients=None):
    species = np.asarray(species)
    coordinates = np.asarray(coordinates, np.float32)
    meta, arrays = _prep(species, coordinates)
    key = (meta["nch"], meta["clo"], meta["chi"],
           meta["nchr"], meta["clo_r"], meta["chi_r"])
    if key not in _CACHE:
        _CACHE[key] = _build(meta["nch"], list(meta["clo"]),
                             list(meta["chi"]), meta["nchr"],
                             list(meta["clo_r"]), list(meta["chi_r"]))
    nc = _CACHE[key]

    ct = _build_consts()
    in_maps = []
    for c in range(NCORES):
        in_maps.append({
            "geo": arrays["geo"][c], "oh": arrays["oh"][c],
            "geor": arrays["geor"][c], "ohr": arrays["ohr"][c],
            "consts": ct,
        })
    res = run_bass_kernel_spmd(nc, in_maps, core_ids=list(range(NCORES)))
    return _unpack(res.results, meta)


# revision 54
# speedup vs baseline: 1.2067x; 1.2067x over previous
"""ANI-style AEV computer (radial + angular) on 8 Trainium2 NeuronCores.

Strategy
--------
Data-parallel over molecules (32/core), with host-side *indexing only*
(neighborlists / triple lists / one-hot bin matrices); every floating-point
operation of the AEV math runs on-device.

Angular part: the host enumerates surviving triples (center i, neighbors
j<k within Rca) into a flat per-core list, sorted by (molecule-slot,
center-half, center, species-pair-bin).  The device computes, per
128-triple chunk column: geometry (vectors, d^2, dot via grouped X-reduce,
j/k batched) -> 1/d and d via ACT ln/exp -> cos/sin of the angle (sin via
ln/exp of 1-cos^2) -> cutoff poly -> f2 = exp(-eta/4 (dsum-2shf)^2) ->
q8 via the angle-sum identity (4 shifts + mirrored 4) -> f1 = q^zeta via
exp(zeta*ln q) -> G = w*f2 (x) f1 [bf16, 32 features], binned into
(center, species-pair) segments by PE matmuls against fp8 one-hot
matrices (PSUM-accumulated across a segment-group's chunks; 5 psum-group
blocks of descending size trail the DVE chain through PE -> ACT copy ->
DMA out).

Radial part: a within-Rcr pair list chunked the same way (groups =
molecule slots, chunk-aligned so every group's chunks are private);
segment = center*4 + species(j), 96 segments padded to a 128-wide fp8
one-hot so LDWEIGHTS uses the fast quadrant path.  Pair matmuls land in
two alternating psum banks (even/odd slots).  The radial shift expansion
rides the ACT slack between the angular activations.

Performance notes: one ACT table set (natural_log_exp) is used - cutoff
cosines are a degree-3 Chebyshev polynomial in u^2 (error ~1e-5), both
cutoffs share one merged Horner chain; outputs are staged/DMAed as bf16
and upcast on host; the GEO DMA streams in two halves ahead of the big
one-hots so the DVE geometry starts as early as possible.
"""

import os
import sys

import numpy as np

for _p in ("/opt/trn_rl_repo", "/root/.axon_site/_ro/trn_rl_repo"):
    if os.path.isdir(_p) and _p not in sys.path:
        sys.path.insert(0, _p)

import concourse.bass as bass
import concourse.mybir as mybir
from concourse import bacc, tile
from concourse.bass_utils import run_bass_kernel_spmd

import ml_dtypes

AF = mybir.ActivationFunctionType
ALU = mybir.AluOpType
AX = mybir.AxisListType
dt = mybir.dt
AP = bass.AP

FP8 = ml_dtypes.float8_e4m3

# ---- hyperparameters (match reference) ----
NCORES = 8
M, A = 256, 24
MLOC = M // NCORES          # 32 molecules per core
RCR, RCA = 5.2, 3.5
ETA_R, ETA_A, ZETA = 16.0, 8.0, 32.0
SHF_R = np.linspace(0.9, 5.2, 17)[:-1].astype(np.float64)   # 16
SHF_A = np.linspace(0.9, 3.5, 5)[:-1].astype(np.float64)    # 4
SHF_Z = (np.arange(8) + 0.5) * np.pi / 8.0                   # 8
NPAIR, RSUB, ASUB = 10, 16, 32
NSEG = 120                  # segments per psum group = 12 centers x 10 bins
GSEG = 128                  # one-hot width (8 pad cols)
NG = 2 * MLOC               # 64 groups/core (2 per molecule slot)
PGRP = 16                   # psum groups packed per PSUM bank tile
RGRP = MLOC // 4            # 8 radial groups of 4 molecules (96 = 4*24 rows)
# device z order: front shifts 0..3, then back shifts (pi - z) = ref 7,6,5,4
ZPERM = np.array([0, 1, 2, 3, 7, 6, 5, 4])

_TRIU = np.zeros((4, 4), np.int64)
_s1, _s2 = np.triu_indices(4)
_TRIU[_s1, _s2] = np.arange(len(_s1))
_TRIU[_s2, _s1] = _TRIU[_s1, _s2]

# ---- degree-3 (in v=u^2) Chebyshev fit of cos(pi*u/2) on u in [0,1]
# (max err ~1.2e-5 -> ~5e-5 on features; negligible vs the bf16 outputs) ----
def _cos_poly():
    v = np.linspace(0.0, 1.0, 4001)
    tgt = np.cos(0.5 * np.pi * np.sqrt(v))
    from numpy.polynomial import chebyshev as C
    ch = C.Chebyshev.fit(v, tgt, 3, domain=[0, 1])
    pw = ch.convert(kind=np.polynomial.Polynomial)
    c = pw.coef  # c0..c3 in v
    K = c[3]
    a = c[:3] / K  # monic residual coeffs a0..a2
    return K, a


_POLY_K, _POLY_A = _cos_poly()

# const tile column map ([128, _C_W] fp32)
_C_SHF2A = 0     # 4  : 2*shf_a
_C_SHFR = 4      # 16 : shf_r
_C_CZH = 20      # 4  : 0.5*cos(shf_z[0:4])
_C_SZH = 24      # 4  : 0.5*sin(shf_z[0:4])
_C_MASK = 28     # 24 : radial i==j mask*100 (valid on partitions 0..95)
_C_ONE = 52      # 1  : 1.0 (ln bias for sin)
_C_F2B = 53      # 1  : angular exp bias ln(2*K^4)
_C_RADB = 54     # 1  : radial exp bias ln(0.25*K^2)
_C_W = 55


def _build_consts():
    ct = np.zeros((128, _C_W), np.float32)
    ct[:, _C_SHF2A:_C_SHF2A + 4] = 2.0 * SHF_A
    ct[:, _C_SHFR:_C_SHFR + 16] = SHF_R
    ct[:, _C_CZH:_C_CZH + 4] = 0.5 * np.cos(SHF_Z[:4])
    ct[:, _C_SZH:_C_SZH + 4] = 0.5 * np.sin(SHF_Z[:4])
    mask = np.zeros((128, 24), np.float32)
    for mb in range(4):
        for j in range(24):
            mask[mb * 24 + j, j] = 100.0
    ct[:, _C_MASK:_C_MASK + 24] = mask
    K = _POLY_K
    ct[:, _C_ONE] = 1.0
    ct[:, _C_F2B] = np.log(2.0) + 4.0 * np.log(abs(K))
    ct[:, _C_RADB] = np.log(0.25) + 2.0 * np.log(abs(K))
    return ct


# ============================================================
# host-side indexing prep (no float math enters the output path)
# ============================================================

def _prep(species, coordinates):
    sp = np.asarray(species)
    co = np.asarray(coordinates, np.float32)
    cod = co.astype(np.float64)
    vec = cod[:, None, :, :] - cod[:, :, None, :]       # [m, i, j, 3] = r_j - r_i
    dmat = np.sqrt(np.maximum((vec ** 2).sum(-1), 0.0))
    adj = (dmat <= RCA) & ~np.eye(A, dtype=bool)[None]

    # per-(m, i) neighbor lists and per-half triple counts
    nbrs = [[np.where(adj[m, i])[0] for i in range(A)] for m in range(M)]
    tri_mi = np.array([[len(nbrs[m][i]) * (len(nbrs[m][i]) - 1) // 2
                        for i in range(A)] for m in range(M)], np.int64)
    Th = np.stack([tri_mi[:, :12].sum(1), tri_mi[:, 12:].sum(1)], 1)  # [M, 2]

    # molecule -> (core, slot): sort by total triples, deal rank-groups of 8
    order = np.argsort(-(Th.sum(1)), kind="stable")
    slot2mol = np.empty((NCORES, MLOC), np.int64)
    for s in range(MLOC):
        for c in range(NCORES):
            slot2mol[c, s] = order[s * NCORES + c]

    # flat per-core triple list (sorted by slot/half/unit/p); no per-group
    # padding -- groups map to chunk RANGES (union over cores), and boundary
    # chunks matmul into more than one psum group.
    SYNCW = 8   # re-align cores to a chunk boundary every SYNCW groups
    tlo = np.zeros((NCORES, NG), np.int64)   # triple range per group
    thi = np.zeros((NCORES, NG), np.int64)
    posv = np.zeros(NCORES, np.int64)
    for g in range(NG):
        s, h = g // 2, g % 2
        if g % SYNCW == 0:
            posv[:] = int(np.ceil(posv.max() / 128.0)) * 128
        tlo[:, g] = posv
        posv += Th[slot2mol[:, s], h]
        thi[:, g] = posv
    nch = int(np.ceil(posv.max() / 128.0))
    # chunk span per group (uniform): union over cores
    clo = np.empty(NG, np.int64)
    chi = np.empty(NG, np.int64)
    for g in range(NG):
        clo[g] = (tlo[:, g] // 128).min()
        hi = np.maximum(thi[:, g] - 1, tlo[:, g]) // 128
        chi[g] = max(hi.max(), clo[g])
    span = (chi - clo + 1).astype(np.int64)
    mm_base = np.concatenate([[0], np.cumsum(span)])
    n_mm = int(mm_base[-1])

    # packed geometry: [:, :, 0:3]=r_j, [:, :, 3:6]=r_k, [:, :, 6:9]=r_i
    geo = np.zeros((NCORES, 128, nch, 9), np.float32)
    oh = np.zeros((NCORES, 128, n_mm, GSEG), FP8)

    for c in range(NCORES):
        def put_pad(a, b, mref):
            if a >= b:
                return
            t_idx = np.arange(a, b)
            chs, ts = t_idx // 128, t_idx % 128
            geo[c, ts, chs, 0:3] = mref + np.array([50, 0, 0], np.float32)
            geo[c, ts, chs, 3:6] = mref + np.array([0, 50, 0], np.float32)
            geo[c, ts, chs, 6:9] = mref
        prev_end = 0
        for s in range(MLOC):
            m = slot2mol[c, s]
            for h in range(2):
                g = 2 * s + h
                put_pad(prev_end, tlo[c, g], co[m, 0])  # sync-pad gap
                pos = tlo[c, g]
                for u in range(12):
                    i = h * 12 + u
                    nb = nbrs[m][i]
                    if len(nb) < 2:
                        continue
                    jj, kk = np.triu_indices(len(nb), 1)
                    j, k = nb[jj], nb[kk]
                    p = _TRIU[sp[m, j], sp[m, k]]
                    o = np.argsort(p, kind="stable")
                    j, k, p = j[o], k[o], p[o]
                    n = len(j)
                    t_idx = np.arange(pos, pos + n)
                    chs, ts = t_idx // 128, t_idx % 128
                    geo[c, ts, chs, 0:3] = co[m, j]
                    geo[c, ts, chs, 3:6] = co[m, k]
                    geo[c, ts, chs, 6:9] = np.broadcast_to(co[m, i], (n, 3))
                    oh[c, ts, mm_base[g] + chs - clo[g], p * 12 + u] = 1
                    pos += n
                prev_end = pos
        put_pad(prev_end, nch * 128, co[slot2mol[c, 0], 0])

    # ---- radial inputs: pair-list (only pairs within Rcr), chunked like
    # the angular triples.  Groups = molecule slots (32); segment within a
    # group = i*4 + species(j) (96 of them); boundary chunks matmul into
    # two psum column ranges.
    adjr = (dmat <= RCR) & ~np.eye(A, dtype=bool)[None]
    prc = adjr.sum((1, 2))                     # pairs per molecule
    tlo_r = np.zeros((NCORES, MLOC), np.int64)
    thi_r = np.zeros((NCORES, MLOC), np.int64)
    posv = np.zeros(NCORES, np.int64)
    SYNCR = 1
    for g in range(MLOC):
        if g % SYNCR == 0:
            posv[:] = int(np.ceil(posv.max() / 128.0)) * 128
        tlo_r[:, g] = posv
        posv += prc[slot2mol[:, g]]
        thi_r[:, g] = posv
    nchr = int(np.ceil(posv.max() / 128.0))
    clo_r = np.empty(MLOC, np.int64)
    chi_r = np.empty(MLOC, np.int64)
    for g in range(MLOC):
        clo_r[g] = (tlo_r[:, g] // 128).min()
        hi = np.maximum(thi_r[:, g] - 1, tlo_r[:, g]) // 128
        chi_r[g] = max(hi.max(), clo_r[g])
    span_r = (chi_r - clo_r + 1).astype(np.int64)
    mmb_r = np.concatenate([[0], np.cumsum(span_r)])
    n_mmr = int(mmb_r[-1])

    geor = np.zeros((NCORES, 128, nchr, 6), np.float32)
    ohr = np.zeros((NCORES, 128, n_mmr, 128), FP8)
    for c in range(NCORES):
        def put_pad_r(a, b, mref):
            if a >= b:
                return
            t_idx = np.arange(a, b)
            chs, ts = t_idx // 128, t_idx % 128
            geor[c, ts, chs, 0:3] = mref + np.array([50, 0, 0], np.float32)
            geor[c, ts, chs, 3:6] = mref
        prev_end = 0
        for g in range(MLOC):
            m = slot2mol[c, g]
            put_pad_r(prev_end, tlo_r[c, g], co[m, 0])
            pos = tlo_r[c, g]
            ii, jj = np.nonzero(adjr[m])
            n = len(ii)
            t_idx = np.arange(pos, pos + n)
            chs, ts = t_idx // 128, t_idx % 128
            geor[c, ts, chs, 0:3] = co[m, jj]
            geor[c, ts, chs, 3:6] = co[m, ii]
            ohr[c, ts, mmb_r[g] + chs - clo_r[g], ii * 4 + sp[m, jj]] = 1
            pos += n
            prev_end = pos
        put_pad_r(prev_end, nchr * 128, co[slot2mol[c, 0], 0])

    meta = dict(nch=nch, n_mm=n_mm, clo=tuple(int(x) for x in clo),
                chi=tuple(int(x) for x in chi), slot2mol=slot2mol,
                nchr=nchr, n_mmr=n_mmr,
                clo_r=tuple(int(x) for x in clo_r),
                chi_r=tuple(int(x) for x in chi_r))
    arrays = dict(geo=geo, oh=oh, geor=geor, ohr=ohr)
    return meta, arrays


# ============================================================
# device program
# ============================================================

def _bb(ap, dims, off=0):
    """Build a broadcast/strided view: keep ap's partition dim, replace free
    dims with explicit [step, count] pairs (element units)."""
    return AP(ap.tensor, ap.offset + off,
              [list(ap.ap[0])] + [list(d) for d in dims])


def _build(nch, clo, chi, nchr, clo_r, chi_r):
    span = [chi[g] - clo[g] + 1 for g in range(NG)]
    mm_base = [0]
    for g in range(NG):
        mm_base.append(mm_base[-1] + span[g])
    n_mm = mm_base[-1]
    nb = nch
    span_r = [chi_r[g] - clo_r[g] + 1 for g in range(MLOC)]
    mmb_r = [0]
    for g in range(MLOC):
        mmb_r.append(mmb_r[-1] + span_r[g])
    n_mmr = mmb_r[-1]
    nbr = nchr

    nc = bacc.Bacc(None, target_bir_lowering=False)
    geo_d = nc.declare_dram_parameter("geo", [128, nch, 9], dt.float32, False)
    oh_d = nc.declare_dram_parameter("oh", [128, n_mm, GSEG], dt.float8e4,
                                     False)
    geor_d = nc.declare_dram_parameter("geor", [128, nchr, 6], dt.float32,
                                       False)
    ohr_d = nc.declare_dram_parameter("ohr", [128, n_mmr, 128], dt.float8e4,
                                      False)
    ct_d = nc.declare_dram_parameter("consts", [128, _C_W], dt.float32, False)
    outa_d = nc.declare_dram_parameter("outa", [GSEG, NG * 32], dt.bfloat16,
                                       True)
    outr_d = nc.declare_dram_parameter("outr", [128, MLOC * 16], dt.bfloat16,
                                       True)

    K, a = _POLY_K, _POLY_A
    # fold 2*K^4 (w = 2*fc_j*fc_k = 2*(K^2 s4j^2)(K^2 s4k^2)) into f2's exp bias
    F2BIAS = float(np.log(2.0) + 4.0 * np.log(abs(K)))
    # radial: rad = 0.25 * fc * exp(...) ; fc = (K*s4)^2
    RADBIAS = float(np.log(0.25) + 2.0 * np.log(abs(K)))
    f32, bf16 = dt.float32, dt.bfloat16

    with tile.TileContext(nc) as tc:
        with (
            tc.tile_pool(name="const", bufs=1) as cpool,
            tc.tile_pool(name="io", bufs=1) as io,
            tc.tile_pool(name="geo", bufs=1) as gp_,
            tc.tile_pool(name="feat", bufs=1) as feat,
            tc.tile_pool(name="stg", bufs=1) as stg,
            tc.tile_pool(name="gt", bufs=5) as gtp,
            tc.tile_pool(name="ps", bufs=5, space="PSUM") as ps,
            tc.tile_pool(name="psr", bufs=1, space="PSUM") as psr,
        ):
            CT = cpool.tile([128, _C_W], dt.float32)

            AZSTG = stg.tile([GSEG, NG * 32], bf16)   # angular staging
            RDSTG = stg.tile([128, MLOC * 16], bf16)  # radial staging

            V = nc.vector
            S = nc.scalar

            # DMA order: angular geometry first (gates the long chain), the
            # radial pair list second, then the big one-hots (matmul phase).
            GEO = io.tile([128, 9 * nch], f32, tag="geo")
            nh0 = nch // 2
            nc.sync.dma_start(
                GEO[:, :9 * nh0].rearrange("p (n c) -> p n c", c=9),
                geo_d[:, :nh0])
            nc.sync.dma_start(
                GEO[:, 9 * nh0:].rearrange("p (n c) -> p n c", c=9),
                geo_d[:, nh0:])
            GEOR = io.tile([128, 6 * nchr], f32, tag="geor")
            nc.scalar.dma_start(
                GEOR[:].rearrange("p (n c) -> p n c", c=6), geor_d[:])
            nc.scalar.dma_start(CT[:], ct_d[:])
            OHR = io.tile([128, 128 * n_mmr], dt.float8e4, tag="ohr")
            nc.sync.dma_start(
                OHR[:].rearrange("p (n s) -> p n s", s=128), ohr_d[:])
            OHL = io.tile([128, GSEG * n_mm], dt.float8e4, tag="oh")
            nc.sync.dma_start(
                OHL[:].rearrange("p (n s) -> p n s", s=GSEG), oh_d[:])

            def poly_fc(E, dist, n, tag, rc, npart=128, stt=True):
                """fc up to factor K^2: returns s4 with fc = (K*s4)^2."""
                u = gp_.tile([npart, n], f32, tag=tag + "_u")
                E.tensor_scalar(u[:], dist[:], rc, 1.0 / rc, ALU.min,
                                ALU.mult)
                v = gp_.tile([npart, n], f32, tag=tag + "_v")
                E.tensor_tensor(v[:], u[:], u[:], ALU.mult)
                acc = gp_.tile([npart, n], f32, tag=tag + "_acc")
                if stt:
                    E.scalar_tensor_tensor(acc[:], v[:], float(a[3]), v[:],
                                           ALU.add, ALU.mult)
                    E.scalar_tensor_tensor(acc[:], acc[:], float(a[2]), v[:],
                                           ALU.add, ALU.mult)
                    E.scalar_tensor_tensor(acc[:], acc[:], float(a[1]), v[:],
                                           ALU.add, ALU.mult)
                else:
                    # gpsimd lacks scalar_tensor_tensor: Horner via TS+TT
                    E.tensor_scalar(acc[:], v[:], float(a[3]), None, ALU.add)
                    E.tensor_tensor(acc[:], acc[:], v[:], ALU.mult)
                    E.tensor_scalar(acc[:], acc[:], float(a[2]), None,
                                    ALU.add)
                    E.tensor_tensor(acc[:], acc[:], v[:], ALU.mult)
                    E.tensor_scalar(acc[:], acc[:], float(a[1]), None,
                                    ALU.add)
                    E.tensor_tensor(acc[:], acc[:], v[:], ALU.mult)
                E.tensor_scalar(acc[:], acc[:], float(a[0]), None, ALU.add)
                return acc

            # ---------------- angular geometry (j/k batched) ---------------
            VJK = gp_.tile([128, 6 * nb], f32, tag="vjk")
            SQ = gp_.tile([128, 6 * nb], f32, tag="sq")
            D2JK = gp_.tile([128, 2 * nb], f32, tag="d2jk")
            DOTP = gp_.tile([128, 3 * nb], f32, tag="dotp")
            DOT = gp_.tile([128, nb], f32, tag="dot")
            V.tensor_tensor(
                _bb(VJK[:], [[6, nb], [3, 2], [1, 3]]),
                _bb(GEO[:], [[9, nb], [3, 2], [1, 3]]),
                _bb(GEO[:], [[9, nb], [0, 2], [1, 3]], off=6),
                ALU.subtract)
            V.tensor_tensor(SQ[:], VJK[:], VJK[:], ALU.mult)
            V.tensor_reduce(D2JK[:], _bb(SQ[:], [[3, 2 * nb], [1, 3]]),
                            AX.X, ALU.add)
            V.tensor_tensor(
                _bb(DOTP[:], [[3, nb], [1, 3]]),
                _bb(VJK[:], [[6, nb], [1, 3]], off=0),
                _bb(VJK[:], [[6, nb], [1, 3]], off=3),
                ALU.mult)
            V.tensor_reduce(DOT[:], _bb(DOTP[:], [[3, nb], [1, 3]]),
                            AX.X, ALU.add)

            # ---- radial pair geometry fills DVE while ACT runs ln/exp ----
            VR = gp_.tile([128, 3 * nbr], f32, tag="vr")
            V.tensor_tensor(
                _bb(VR[:], [[3, nbr], [1, 3]]),
                _bb(GEOR[:], [[6, nbr], [1, 3]], off=0),
                _bb(GEOR[:], [[6, nbr], [1, 3]], off=3),
                ALU.subtract)
            S.activation(VR[:], VR[:], AF.Square)
            RD2 = gp_.tile([128, nbr], f32, tag="rd2")
            V.tensor_reduce(RD2[:], _bb(VR[:], [[3, nbr], [1, 3]]),
                            AX.X, ALU.add)

            # d, 1/d via single ln + two exps (one ACT table set total)
            LNJK = gp_.tile([128, 2 * nb], f32, tag="lnjk")
            DJK = gp_.tile([128, 2 * nb], f32, tag="djk")
            RJK = gp_.tile([128, 2 * nb], f32, tag="rjk")
            S.activation(LNJK[:], D2JK[:], AF.Ln)
            S.activation(DJK[:], LNJK[:], AF.Exp, scale=0.5)
            S.activation(RJK[:], LNJK[:], AF.Exp, scale=-0.5)
            RD = gp_.tile([128, nbr], f32, tag="rdist")
            S.activation(RD[:], RD2[:], AF.Ln)
            S.activation(RD[:], RD[:], AF.Exp, scale=0.5)

            COS = gp_.tile([128, nb], f32, tag="cos")
            V.scalar_tensor_tensor(COS[:], DOT[:], 0.95,
                                   _bb(RJK[:], [[2, nb]], off=0),
                                   ALU.mult, ALU.mult)
            V.tensor_tensor(COS[:], COS[:],
                            _bb(RJK[:], [[2, nb]], off=1), ALU.mult)
            CS2 = gp_.tile([128, nb], f32, tag="cs2")
            S.activation(CS2[:], COS[:], AF.Square)
            SIN = gp_.tile([128, nb], f32, tag="sin")
            # sin = exp(0.5*ln(1 - cos^2))
            S.activation(SIN[:], CS2[:], AF.Ln, scale=-1.0,
                         bias=CT[:, _C_ONE:_C_ONE + 1])
            S.activation(SIN[:], SIN[:], AF.Exp, scale=0.5)

            # radial shift expansion in two chunk-halves (split at the
            # group-16 chunk boundary) so DVE/ACT/PE pipeline
            GH = MLOC // 2
            hc0 = chi_r[GH - 1] + 1
            RT = feat.tile([128, 16 * nbr], f32, tag="rt")

            def rad_rt(h):
                lo = 0 if h == 0 else hc0
                hi = hc0 if h == 0 else nbr
                V.tensor_tensor(
                    _bb(RT[:], [[16, hi - lo], [1, 16]], off=16 * lo),
                    _bb(RD[:], [[1, hi - lo], [0, 16]], off=lo),
                    _bb(CT[:, _C_SHFR:], [[0, hi - lo], [1, 16]]),
                    ALU.subtract)
                hs = slice(16 * lo, 16 * hi)
                S.activation(RT[:, hs], RT[:, hs], AF.Square)
                S.activation(RT[:, hs], RT[:, hs], AF.Exp, scale=-ETA_R,
                             bias=CT[:, _C_RADB:_C_RADB + 1])

            # radial square/exp fills the ACT slack before the angular
            # feature activations
            rad_rt(0)
            rad_rt(1)

            # merged cutoff poly: u for both cutoffs side by side, then one
            # Horner chain over [2nb + nbr] (5 big ops instead of 10)
            NPOLY = 2 * nb + nbr
            PU = gp_.tile([128, NPOLY], f32, tag="pu")
            V.tensor_scalar(PU[:, :2 * nb], DJK[:], RCA, 1.0 / RCA, ALU.min,
                            ALU.mult)
            V.tensor_scalar(PU[:, 2 * nb:], RD[:], RCR, 1.0 / RCR, ALU.min,
                            ALU.mult)
            PV = gp_.tile([128, NPOLY], f32, tag="pv")
            V.tensor_tensor(PV[:], PU[:], PU[:], ALU.mult)
            PA = gp_.tile([128, NPOLY], f32, tag="pa")
            V.scalar_tensor_tensor(PA[:], PV[:], float(a[2]), PV[:],
                                   ALU.add, ALU.mult)
            V.scalar_tensor_tensor(PA[:], PA[:], float(a[1]), PV[:],
                                   ALU.add, ALU.mult)
            V.tensor_scalar(PA[:], PA[:], float(a[0]), None, ALU.add)
            S4 = PA[:, :2 * nb]
            RFC = gp_.tile([128, nbr], f32, tag="rfc")
            V.tensor_tensor(RFC[:], PA[:, 2 * nb:], PA[:, 2 * nb:],
                            ALU.mult)
            RAD = feat.tile([128, 16 * nbr], bf16, tag="rad")

            def rad_fill(h):
                lo = 0 if h == 0 else hc0
                hi = hc0 if h == 0 else nbr
                V.tensor_tensor(
                    _bb(RAD[:], [[16, hi - lo], [1, 16]], off=16 * lo),
                    _bb(RFC[:], [[1, hi - lo], [0, 16]], off=lo),
                    _bb(RT[:], [[16, hi - lo], [1, 16]], off=16 * lo),
                    ALU.mult)

            rad_fill(0)
            rad_fill(1)
            W2 = gp_.tile([128, nb], f32, tag="w2")
            V.tensor_tensor(W2[:], _bb(PA[:], [[2, nb]], off=0),
                            _bb(PA[:], [[2, nb]], off=1), ALU.mult)
            V.tensor_tensor(W2[:], W2[:], W2[:], ALU.mult)

            USUM = gp_.tile([128, nb], f32, tag="usum")
            V.tensor_tensor(USUM[:], _bb(DJK[:], [[2, nb]], off=0),
                            _bb(DJK[:], [[2, nb]], off=1), ALU.add)

            # f2[a] = exp(-eta/4*(u - 2shf_a)^2 + F2BIAS), layout (n, a)
            T4 = feat.tile([128, 4 * nb], f32, tag="t4")
            V.tensor_tensor(
                _bb(T4[:], [[4, nb], [1, 4]]),
                _bb(USUM[:], [[1, nb], [0, 4]]),
                _bb(CT[:, _C_SHF2A:], [[0, nb], [1, 4]]),
                ALU.subtract)
            S.activation(T4[:], T4[:], AF.Square)
            S.activation(T4[:], T4[:], AF.Exp, scale=-ETA_A / 4.0,
                         bias=CT[:, _C_F2B:_C_F2B + 1])
            WF2 = feat.tile([128, 4 * nb], bf16, tag="wf2")
            V.tensor_tensor(
                _bb(WF2[:], [[4, nb], [1, 4]]),
                _bb(W2[:], [[1, nb], [0, 4]]),
                _bb(T4[:], [[4, nb], [1, 4]]),
                ALU.mult)

            # q8: q_z = 0.5 + cos*czh_z + sin*szh_z for z=0..3 and the
            # mirrored shifts (pi - z): 0.5 - cos*czh_z + sin*szh_z
            A4 = feat.tile([128, 4 * nb], f32, tag="a4")
            B4 = feat.tile([128, 4 * nb], f32, tag="b4")
            V.tensor_tensor(
                _bb(A4[:], [[4, nb], [1, 4]]),
                _bb(COS[:], [[1, nb], [0, 4]]),
                _bb(CT[:, _C_CZH:], [[0, nb], [1, 4]]),
                ALU.mult)
            V.tensor_tensor(
                _bb(B4[:], [[4, nb], [1, 4]]),
                _bb(SIN[:], [[1, nb], [0, 4]]),
                _bb(CT[:, _C_SZH:], [[0, nb], [1, 4]]),
                ALU.mult)
            Q8 = feat.tile([128, 8 * nb], f32, tag="q8")
            V.scalar_tensor_tensor(
                _bb(Q8[:], [[8, nb], [1, 4]], off=0),
                _bb(A4[:], [[4, nb], [1, 4]]), 0.5,
                _bb(B4[:], [[4, nb], [1, 4]]),
                ALU.add, ALU.add)
            V.scalar_tensor_tensor(
                _bb(Q8[:], [[8, nb], [1, 4]], off=4),
                _bb(B4[:], [[4, nb], [1, 4]]), 0.5,
                _bb(A4[:], [[4, nb], [1, 4]]),
                ALU.add, ALU.subtract)
            # split ln/exp at the first psum-block chunk boundary so the
            # first Gt build can start before the full-width exp finishes
            hcq = 8 * (chi[PGRP - 1] + 1 - clo[0])
            F1 = feat.tile([128, 8 * nb], bf16, tag="f1")
            S.activation(Q8[:, :hcq], Q8[:, :hcq], AF.Ln)
            S.activation(F1[:, :hcq], Q8[:, :hcq], AF.Exp, scale=float(ZETA))
            S.activation(Q8[:, hcq:], Q8[:, hcq:], AF.Ln)
            S.activation(F1[:, hcq:], Q8[:, hcq:], AF.Exp, scale=float(ZETA))

            # ---- angular binning + radial expansion, interleaved so the
            # radial DVE/ACT work pipelines with Gt builds / PE / copies ----
            c0 = clo[0]
            gwmax = max(chi[min(gt + PGRP, NG) - 1] - clo[gt] + 1
                        for gt in range(0, NG, PGRP))
            # two psum banks (even/odd groups) + round-robin chunk order so
            # consecutive matmuls never hit the same accumulation region
            rptA = psr.tile([128, MLOC * 8], dt.float32, tag="rpsA")
            rptB = psr.tile([128, MLOC * 8], dt.float32, tag="rpsB")

            def rad_mm(h):
                for g in range(h * GH, (h + 1) * GH):
                    rp = rptA if g % 2 == 0 else rptB
                    q = 16 * (g // 2)
                    for k in range(span_r[g]):
                        cc = clo_r[g] + k
                        nc.tensor.matmul(
                            rp[:, q:q + 16],
                            OHR[:, 128 * (mmb_r[g] + k):
                                 128 * (mmb_r[g] + k + 1)],
                            RAD[:, 16 * cc:16 * (cc + 1)],
                            start=(k == 0), stop=(k == span_r[g] - 1))
                if h == 1:
                    S.activation(RDSTG[:, :MLOC * 8], rptA[:], AF.Copy)
                    S.activation(RDSTG[:, MLOC * 8:], rptB[:], AF.Copy)
                    nc.sync.dma_start(outr_d[:], RDSTG[:])

            def gt_block(gt, glen=PGRP):
                gl = min(gt + glen, NG)
                ca, cb = clo[gt] - c0, chi[gl - 1] + 1 - c0
                Gt = gtp.tile([128, 32 * gwmax], bf16, tag="G")
                V.tensor_tensor(
                    _bb(Gt[:], [[32, cb - ca], [8, 4], [1, 8]]),
                    _bb(WF2[:, 4 * ca:], [[4, cb - ca], [1, 4], [0, 8]]),
                    _bb(F1[:, 8 * ca:], [[8, cb - ca], [0, 4], [1, 8]]),
                    ALU.mult)
                pt = ps.tile([GSEG, 32 * PGRP], dt.float32, tag="ps")
                pt = pt[:, :32 * glen]
                for g in range(gt, gl):
                    gi = g - gt
                    for k in range(span[g]):
                        cc = clo[g] + k           # absolute chunk
                        nc.tensor.matmul(
                            pt[:, 32 * gi:32 * (gi + 1)],
                            OHL[:, GSEG * (mm_base[g] + k):
                                  GSEG * (mm_base[g] + k + 1)],
                            Gt[:, 32 * (cc - c0 - ca):
                                  32 * (cc - c0 - ca + 1)],
                            start=(k == 0), stop=(k == span[g] - 1))
                sl = slice(32 * gt, 32 * gl)
                S.activation(AZSTG[:, sl], pt[:], AF.Copy)
                nc.sync.dma_start(outa_d[:, sl], AZSTG[:, sl])

            rad_mm(0)
            rad_mm(1)
            gt_block(0)
            gt_block(16)
            gt_block(32, 16)
            gt_block(48, 10)
            gt_block(58, 6)

    _patch_act_tables()
    nc.compile()
    return nc


_ACT_PATCHED = False


def _patch_act_tables():
    """Make Ln/Exp resolve only to the combined natural_log_exp set, so the
    table-load pass emits ONE load instead of thrashing between the ln-only
    and exp-only sets (1.28us per reload)."""
    global _ACT_PATCHED
    if _ACT_PATCHED:
        return
    orig = bacc.get_activation_tables

    def patched(arch):
        t = dict(orig(arch))
        out = {}
        for name, fns in t.items():
            if name != "natural_log_exp_and_others":
                fns = {f for f in fns if f not in (AF.Ln, AF.Exp)}
            out[name] = fns
        return out

    bacc.get_activation_tables = patched
    _ACT_PATCHED = True


_CACHE = {}


def _unpack(res, meta):
    out = np.empty((M, A, 384), np.float32)
    for c in range(NCORES):
        outa = np.asarray(res[c]["outa"]).astype(np.float32)  # [128, NG*32]
        outr = np.asarray(res[c]["outr"]).astype(np.float32)[:96]
        ang = outa.reshape(GSEG, NG, 32)[:120]
        ang = ang.reshape(10, 12, MLOC, 2, 4, 8)        # [p, u, s, h, a, zd]
        ang = ang[..., ZPERM]                           # device z -> ref z
        ang = ang.transpose(2, 3, 1, 0, 4, 5).reshape(MLOC, A, 320)
        # outr rows: seg = i*4 + species(j); cols: [even-slot bank | odd-slot
        # bank], each bank col = (slot//2)*16 + shf
        rad = outr.reshape(A, 4, 2, MLOC // 2, 16)      # [i, sp, ab, gi, r]
        rad = rad.transpose(3, 2, 0, 1, 4).reshape(MLOC, A, 64)
        out[meta["slot2mol"][c], :, :64] = rad
        out[meta["slot2mol"][c], :, 64:] = ang
    return out


def kernel(species, coordinates, coefficients=None):
    species = np.asarray(species)
    coordinates = np.asarray(coordinates, np.float32)
    meta, arrays = _prep(species, coordinates)
    key = (meta["nch"], meta["clo"], meta["chi"],
           meta["nchr"], meta["clo_r"], meta["chi_r"])
    if key not in _CACHE:
        _CACHE[key] = _build(meta["nch"], list(meta["clo"]),
                             list(meta["chi"]), meta["nchr"],
                             list(meta["clo_r"]), list(meta["chi_r"]))
    nc = _CACHE[key]

    ct = _build_consts()
    in_maps = []
    for c in range(NCORES):
        in_maps.append({
            "geo": arrays["geo"][c], "oh": arrays["oh"][c],
            "geor": arrays["geor"][c], "ohr": arrays["ohr"][c],
            "consts": ct,
        })
    res = run_bass_kernel_spmd(nc, in_maps, core_ids=list(range(NCORES)))
    return _unpack(res.results, meta)
